# revision 42
# baseline (speedup 1.0000x reference)
"""Causal self-attention (B=4, T=2048, C=1024, H=16, D=64) on 8 TRN2 cores.

Sharding: core c handles batch b = c//2 and head-group g = c%2 (8 heads).
Each core computes qkv projection for its heads, causal flash attention,
and a partial out-projection (row-parallel over its heads' slice of w_out).
Host sums the two partials per batch and adds biases that commute
(b_k drops under softmax; b_v/b_out fold into a host-side constant row).

Device layout notes:
- x is passed pre-transposed (xT [C, T]) and pre-cast to bf16 by the host,
  so the PE contracts along C with no on-device transposes.
- Q^T/K^T are produced in [j, t] layout (j = head-pair dim: head A d in
  partitions 0:64, head B in 64:128), which is exactly the lhsT/rhs layout
  the attention matmuls need.
- S^T strips [tk=128, tq=512] are computed per head-pair with row-tiled
  (tile_position) K=64 matmuls, exp'd on ACT (no max-subtraction: |S/8|
  stays < ~6 so exp is safe), P@V col-tiled back into one [128, 512] psum
  (head A -> partitions 0:64, head B -> 64:128), rowsums via ones-lhsT
  matmuls into partitions {0, 32} of an L psum, normalization via a
  selector-matmul partition-broadcast of 1/L.
"""

import os
import tempfile
from contextlib import ExitStack

import numpy as np
import ml_dtypes

import concourse.bass as bass
import concourse.tile as tile
from concourse import bacc, mybir
from concourse.bass_utils import run_bass_kernel_spmd

BF16 = mybir.dt.bfloat16
F32 = mybir.dt.float32
AF = mybir.ActivationFunctionType
ALU = mybir.AluOpType

B, T, C, H, D = 4, 2048, 1024, 16, 64
HPG = 8                 # heads per core (group)
PAIRS = HPG // 2        # head pairs per core
GW = HPG * D            # 512: group width of q/k/v
CC = C // 128           # 8 contraction chunks
TT = T // 128           # 16 t-tiles
QC = T // 512           # 4 query chunks of 512
SCALE = 1.0 / np.sqrt(D)
N_CORES = 8


def build_kernel():
    nc = bacc.Bacc("TRN2", target_bir_lowering=False, debug=False,
                   num_devices=N_CORES)
    xT = nc.dram_tensor("xT", [C, T], BF16, kind="ExternalInput").ap()
    wq = nc.dram_tensor("wq", [C, GW], BF16, kind="ExternalInput").ap()
    wk = nc.dram_tensor("wk", [C, GW], BF16, kind="ExternalInput").ap()
    wv = nc.dram_tensor("wv", [C, GW], BF16, kind="ExternalInput").ap()
    bq = nc.dram_tensor("bq", [GW], F32, kind="ExternalInput").ap()
    wo = nc.dram_tensor("wo", [GW, C], BF16, kind="ExternalInput").ap()
    out = nc.dram_tensor("out", [T, C], F32, kind="ExternalOutput").ap()

    with tile.TileContext(nc) as tc, ExitStack() as ctx:
        const_p = ctx.enter_context(tc.tile_pool(name="const", bufs=1))
        w_p = ctx.enter_context(tc.tile_pool(name="w", bufs=1))
        x_p = ctx.enter_context(tc.tile_pool(name="x", bufs=1))
        qk_p = ctx.enter_context(tc.tile_pool(name="qk", bufs=1))
        v_p = ctx.enter_context(tc.tile_pool(name="v", bufs=1))
        y_p = ctx.enter_context(tc.tile_pool(name="y", bufs=1))
        exp_p = ctx.enter_context(tc.tile_pool(name="expt", bufs=8))
        misc_p = ctx.enter_context(tc.tile_pool(name="misc", bufs=2))
        stage_p = ctx.enter_context(tc.tile_pool(name="stage", bufs=3))
        # PSUM budget (8 banks): s_ps 2x[128,1024]=4, small_ps 4x[128,512]=4
        s_ps = ctx.enter_context(tc.tile_pool(name="s_ps", bufs=2, space="PSUM"))
        small_ps = ctx.enter_context(
            tc.tile_pool(name="small_ps", bufs=4, space="PSUM"))
        rec_p = ctx.enter_context(tc.tile_pool(name="rec", bufs=4))
        bc_pool = ctx.enter_context(tc.tile_pool(name="bc_sb", bufs=4))

        def act_recip(out_ap, in_ap):
            eng = nc.scalar
            ins = [eng.lower_ap(in_ap)]
            for val in (0.0, 1.0, 0.0):  # bias, scale, alpha
                ins.append(mybir.ImmediateValue(dtype=mybir.dt.float32,
                                                value=val))
            return eng.add_instruction(mybir.InstActivation(
                name=nc.get_next_instruction_name(),
                func=AF.Reciprocal, ins=ins, outs=[eng.lower_ap(out_ap)]))

        # ---- constants ----
        # selector for broadcasting recip rowsums via PE: row64 (head A's L)
        # -> out partitions 0:64, row63 (head B's L) -> out partitions 64:128
        sel_sb = const_p.tile([65, 128], BF16)
        nc.gpsimd.memset(sel_sb[:], 0.0)
        nc.gpsimd.memset(sel_sb[64:65, 0:64], 1.0)
        nc.gpsimd.memset(sel_sb[32:33, 64:128], 1.0)
        # causal 0/1 mask with column offset 512: mask[p, j] = 1 iff j-512 >= p.
        # Diagonal strip r uses slice [512 - r*128, 1024 - r*128):
        # mask[p, 512 - r*128 + jq] = 1 iff jq >= r*128 + p.
        mask_sb = const_p.tile([128, 1024], BF16)
        nc.gpsimd.memset(mask_sb[:], 1.0)
        nc.gpsimd.affine_select(
            out=mask_sb[:], in_=mask_sb[:], compare_op=ALU.is_ge,
            fill=0.0, base=-512, pattern=[[1, 1024]], channel_multiplier=-1)


        # ---- weight + input loads ----
        # Interleaved per-cc and split along t so the first projection
        # matmul (needs wq[cc0] + xT[cc0, 0:512] only) unblocks within a
        # few microseconds instead of waiting for all ~11 MB of loads.
        wq_sb = w_p.tile([128, CC, GW], BF16)
        wk_sb = w_p.tile([128, CC, GW], BF16)
        wv_sb = w_p.tile([128, CC, GW], BF16)
        xT_sb = x_p.tile([128, CC, T], BF16)
        for cc in range(CC):
            csl = slice(cc * 128, (cc + 1) * 128)
            # spread the critical-path chunks (wq + xT feed the first
            # projections) across BOTH HWDGE issue queues
            (nc.sync if cc % 2 == 0 else nc.scalar).dma_start(
                wq_sb[:, cc, :], wq[csl, :])
            for tc_ in range(QC):
                tsl = slice(tc_ * 512, (tc_ + 1) * 512)
                (nc.sync if tc_ % 2 == 0 else nc.scalar).dma_start(
                    xT_sb[:, cc, tsl], xT[csl, tsl])
            (nc.scalar if cc % 2 == 0 else nc.sync).dma_start(
                wk_sb[:, cc, :], wk[csl, :])
            (nc.scalar if cc % 2 == 0 else nc.sync).dma_start(
                wv_sb[:, cc, :], wv[csl, :])
        wo_sb = w_p.tile([128, PAIRS, C], BF16)
        for jp in range(PAIRS):
            nc.scalar.dma_start(wo_sb[:, jp, :], wo[jp * 128:(jp + 1) * 128, :])
        bq_sb = w_p.tile([128, PAIRS], F32)
        nc.sync.dma_start(bq_sb[:], bq.rearrange("(p c) -> c p", c=128))

        # ---- Q^T / K^T projections: qT[pair] = [128 (2 heads x d), T] ----
        qT = [qk_p.tile([128, T], BF16, tag=f"qT{p}", name=f"qT{p}") for p in range(PAIRS)]
        kT = [qk_p.tile([128, T], BF16, tag=f"kT{p}", name=f"kT{p}") for p in range(PAIRS)]
        for p in range(PAIRS):
            jsl = slice(p * 128, (p + 1) * 128)
            for w_sb, dst, biased in ((wq_sb, qT[p], True), (wk_sb, kT[p], False)):
                # cc-outer with 4 live psum banks so the stationary (weights)
                # is reused across 4 matmuls
                pss = [small_ps.tile([128, 512], F32, tag="sm",
                                     name=f"pp{p}{biased}{i}") for i in range(QC)]
                for cc in range(CC):
                    for t in range(QC):
                        nc.tensor.matmul(pss[t][:], w_sb[:, cc, jsl],
                                         xT_sb[:, cc, t * 512:(t + 1) * 512],
                                         start=(cc == 0), stop=(cc == CC - 1))
                for t in range(QC):
                    tsl = slice(t * 512, (t + 1) * 512)
                    if biased:
                        nc.scalar.activation(dst[:, tsl], pss[t][:],
                                             AF.Identity,
                                             bias=bq_sb[:, p:p + 1])
                    else:
                        nc.scalar.copy(dst[:, tsl], pss[t][:])

        # ---- V projection into ones-augmented layout ----
        # v_sb [128 (tk), tt, h, 128]:
        #   even head (A): cols 0:64 = V, col 64 = 1, rest 0   -> lhsT [.., 0:65]
        #   odd head (B):  col 32 = 1, cols 64:128 = V, rest 0 -> lhsT [.., 0:128]
        # The P@V matmul then lands y_A in psum partitions 0:64 + L_A at 64,
        # and y_B at 64:128 + L_B at 32 (zero-pad columns cost nothing extra:
        # matmul time is set by the streamed N, not M; L_B sits at 32 so the
        # DVE reciprocal ops stay 32-partition-aligned).
        v_sb = v_p.tile([128, TT, HPG, 128], BF16)
        nc.gpsimd.memset(v_sb[:], 0.0)
        for h in range(HPG):
            col = 64 if h % 2 == 0 else 32
            nc.gpsimd.memset(v_sb[:, :, h, col:col + 1], 1.0)
        for tt in range(TT):
            ps = small_ps.tile([128, 512], F32, tag="sm")
            for cc in range(CC):
                nc.tensor.matmul(ps[:], xT_sb[:, cc, tt * 128:(tt + 1) * 128],
                                 wv_sb[:, cc, :],
                                 start=(cc == 0), stop=(cc == CC - 1))
            psv = ps.rearrange("p (h d) -> p h d", d=D)
            nc.scalar.copy(v_sb[:, tt, 0:HPG:2, 0:D], psv[:, 0:HPG:2, :])
            nc.scalar.copy(v_sb[:, tt, 1:HPG:2, 64:128], psv[:, 1:HPG:2, :])

        # ---- attention + out-projection, per query chunk ----
        yT = [y_p.tile([128, T], BF16, tag=f"yT{p}", name=f"yT{p}") for p in range(PAIRS)]
        for qc in range(QC):
            qsl = slice(qc * 512, (qc + 1) * 512)
            for p in range(PAIRS):
                hA, hB = 2 * p, 2 * p + 1
                nki = 4 * qc + 4
                ya_ps = small_ps.tile([128, 512], F32, tag="sm")
                yb_ps = small_ps.tile([128, 512], F32, tag="sm")
                for ki in range(nki):
                    ksl = slice(ki * 128, (ki + 1) * 128)
                    r = ki - 4 * qc
                    off = max(0, r) * 128  # first valid tq col of this strip
                    qv = slice(qc * 512 + off, (qc + 1) * 512)
                    sps = s_ps.tile([128, 1024], F32, tag="s")
                    nc.tensor.matmul(sps[:, off:512], kT[p][0:64, ksl],
                                     qT[p][0:64, qv], start=True, stop=True,
                                     tile_position=(0, 0))
                    nc.tensor.matmul(sps[:, 512 + off:1024], kT[p][64:128, ksl],
                                     qT[p][64:128, qv], start=True, stop=True,
                                     tile_position=(64, 0))
                    et = exp_p.tile([128, 1024], BF16, tag="e")
                    if off == 0:
                        nc.scalar.activation(et[:], sps[:], AF.Exp, scale=SCALE)
                    else:
                        nc.scalar.activation(et[:, off:512], sps[:, off:512],
                                             AF.Exp, scale=SCALE)
                        nc.scalar.activation(et[:, 512 + off:1024],
                                             sps[:, 512 + off:1024],
                                             AF.Exp, scale=SCALE)
                    if r >= 0:
                        # triangular boundary lives in cols [off, off+128);
                        # cols beyond off+128 are fully valid
                        msl = mask_sb[:, 512:640]
                        nc.gpsimd.tensor_mul(et[:, off:off + 128],
                                             et[:, off:off + 128], msl)
                        nc.gpsimd.tensor_mul(et[:, 512 + off:640 + off],
                                             et[:, 512 + off:640 + off], msl)
                    mmkw = dict(start=(ki == 0), stop=(ki == nki - 1),
                                skip_group_check=True)
                    nc.tensor.matmul(ya_ps[:, off:512],
                                     v_sb[:, ki, hA, :],
                                     et[:, off:512], tile_position=(0, 0),
                                     **mmkw)
                    nc.tensor.matmul(yb_ps[:, off:512], v_sb[:, ki, hB, :],
                                     et[:, 512 + off:1024],
                                     tile_position=(0, 0), **mmkw)
                # Release the y psum banks FAST (raw copies + L-row copies),
                # then normalize yT in place off the critical path. The two
                # K=1 selector matmuls broadcast L_A to partitions 0:64 and
                # L_B to 64:128 reading only the rows we wrote.
                rec_t = rec_p.tile([65, 512], BF16, tag="rec")
                nc.vector.tensor_copy(rec_t[64:65, :], ya_ps[64:65, :])
                nc.vector.tensor_copy(rec_t[32:33, :], yb_ps[32:33, :])
                nc.vector.tensor_copy(yT[p][0:64, qsl], ya_ps[0:64, :])
                nc.vector.tensor_copy(yT[p][64:128, qsl], yb_ps[64:128, :])
                bc_ps = small_ps.tile([128, 512], F32, tag="sm")
                nc.tensor.matmul(bc_ps[:], sel_sb[64:65, :], rec_t[64:65, :],
                                 start=True, stop=False, skip_group_check=True)
                nc.tensor.matmul(bc_ps[:], sel_sb[32:33, :], rec_t[32:33, :],
                                 start=False, stop=True, skip_group_check=True)
                bc_sb = bc_pool.tile([128, 512], F32, tag="bc")
                nc.vector.reciprocal(bc_sb[:], bc_ps[:])
                nc.gpsimd.tensor_mul(yT[p][0:64, qsl], yT[p][0:64, qsl],
                                      bc_sb[0:64, :])
                nc.gpsimd.tensor_mul(yT[p][64:128, qsl], yT[p][64:128, qsl],
                                      bc_sb[64:128, :])
            # out-projection for this query chunk
            for tt in range(4 * qc, 4 * qc + 4):
                st = stage_p.tile([128, 1024], F32, tag="st")
                for nck in range(2):
                    nsl = slice(nck * 512, (nck + 1) * 512)
                    ops = small_ps.tile([128, 512], F32, tag="sm")
                    for jp in range(PAIRS):
                        nc.tensor.matmul(
                            ops[:], yT[jp][:, tt * 128:(tt + 1) * 128],
                            wo_sb[:, jp, nsl],
                            start=(jp == 0), stop=(jp == PAIRS - 1))
                    nc.vector.tensor_copy(st[:, nsl], ops[:])
                nc.sync.dma_start(out[tt * 128:(tt + 1) * 128, :], st[:])

    nc.compile()
    return nc


_NC_CACHE = None


def _get_nc():
    global _NC_CACHE
    if _NC_CACHE is None:
        _NC_CACHE = build_kernel()
    return _NC_CACHE


def _shard(x, w_qkv, b_qkv, w_out, b_out):
    """Build per-core input maps. Core c: batch c//2, head-group c%2."""
    bf = ml_dtypes.bfloat16
    in_maps = []
    for c in range(N_CORES):
        b, g = divmod(c, 2)
        gs = slice(g * GW, g * GW + GW)
        in_maps.append({
            "xT": np.ascontiguousarray(x[b].T).astype(bf),
            "wq": np.ascontiguousarray(w_qkv[:, gs]).astype(bf),
            "wk": np.ascontiguousarray(w_qkv[:, C + g * GW:C + g * GW + GW]).astype(bf),
            "wv": np.ascontiguousarray(w_qkv[:, 2 * C + g * GW:2 * C + g * GW + GW]).astype(bf),
            "bq": np.ascontiguousarray(b_qkv[gs]).astype(np.float32),
            "wo": np.ascontiguousarray(w_out[g * GW:g * GW + GW, :]).astype(bf),
        })
    return in_maps


def _unshard(results, b_qkv, w_out, b_out):
    # host-side constant: b_v @ w_out rows (exact: softmax rows sum to 1)
    bv = b_qkv[2 * C:3 * C].astype(np.float64)
    const_row = (bv @ w_out.astype(np.float64)) + b_out.astype(np.float64)
    out = np.empty((B, T, C), dtype=np.float32)
    for b in range(B):
        acc = (results[2 * b]["out"].astype(np.float64)
               + results[2 * b + 1]["out"].astype(np.float64) + const_row)
        out[b] = acc.astype(np.float32)
    return out


def _run(in_maps, trace=False, tmpdir=None):
    nc = _get_nc()
    return run_bass_kernel_spmd(nc, in_maps, core_ids=list(range(N_CORES)),
                                trace=trace, tmpdir=tmpdir)


def kernel(x, w_qkv, b_qkv, w_out, b_out):
    x = np.asarray(x, dtype=np.float32)
    w_qkv = np.asarray(w_qkv, dtype=np.float32)
    b_qkv = np.asarray(b_qkv, dtype=np.float32)
    w_out = np.asarray(w_out, dtype=np.float32)
    b_out = np.asarray(b_out, dtype=np.float32)
    res = _run(_shard(x, w_qkv, b_qkv, w_out, b_out))
    return _unshard(res.results, b_qkv, w_out, b_out)


def kernel_profiled(x, w_qkv, b_qkv, w_out, b_out, tmpdir=None):
    """Like kernel() but captures an NTFF profile (requires the NTFF hook
    to be registered, e.g. via prof_shim.install()). Returns (out, result)."""
    if tmpdir is None:
        tmpdir = tempfile.mkdtemp(prefix="attn_trace_")
    x = np.asarray(x, dtype=np.float32)
    w_qkv = np.asarray(w_qkv, dtype=np.float32)
    b_qkv = np.asarray(b_qkv, dtype=np.float32)
    w_out = np.asarray(w_out, dtype=np.float32)
    b_out = np.asarray(b_out, dtype=np.float32)
    res = _run(_shard(x, w_qkv, b_qkv, w_out, b_out), trace=True,
               tmpdir=tmpdir)
    return _unshard(res.results, b_qkv, w_out, b_out), res


# revision 44
# speedup vs baseline: 1.0920x; 1.0920x over previous
"""Causal self-attention (B=4, T=2048, C=1024, H=16, D=64) on 8 TRN2 cores.

Sharding: core c handles batch b = c//2 and head-group g = c%2 (8 heads).
Each core computes qkv projection for its heads, causal flash attention,
and a partial out-projection (row-parallel over its heads' slice of w_out).
Host sums the two partials per batch and adds biases that commute
(b_k drops under softmax; b_v/b_out fold into a host-side constant row).

Device layout notes:
- x is passed pre-transposed (xT [C, T]) and pre-cast to bf16 by the host,
  so the PE contracts along C with no on-device transposes.
- Q^T/K^T are produced in [j, t] layout (j = head-pair dim: head A d in
  partitions 0:64, head B in 64:128), which is exactly the lhsT/rhs layout
  the attention matmuls need.
- S^T strips [tk=128, tq=512] are computed per head-pair with row-tiled
  (tile_position) K=64 matmuls, exp'd on ACT (no max-subtraction: |S/8|
  stays < ~6 so exp is safe), P@V col-tiled back into one [128, 512] psum
  (head A -> partitions 0:64, head B -> 64:128), rowsums via ones-lhsT
  matmuls into partitions {0, 32} of an L psum, normalization via a
  selector-matmul partition-broadcast of 1/L.
"""

import os
import tempfile
from contextlib import ExitStack

import numpy as np
import ml_dtypes

import concourse.bass as bass
import concourse.tile as tile
from concourse import bacc, mybir
from concourse.bass_utils import run_bass_kernel_spmd

BF16 = mybir.dt.bfloat16
F32 = mybir.dt.float32
AF = mybir.ActivationFunctionType
ALU = mybir.AluOpType

B, T, C, H, D = 4, 2048, 1024, 16, 64
HPG = 8                 # heads per core (group)
PAIRS = HPG // 2        # head pairs per core
GW = HPG * D            # 512: group width of q/k/v
CC = C // 128           # 8 contraction chunks
TT = T // 128           # 16 t-tiles
QC = T // 512           # 4 query chunks of 512
SCALE = 1.0 / np.sqrt(D)
N_CORES = 8


def build_kernel():
    nc = bacc.Bacc("TRN2", target_bir_lowering=False, debug=False,
                   num_devices=N_CORES)
    xT = nc.dram_tensor("xT", [C, T], BF16, kind="ExternalInput").ap()
    wq = nc.dram_tensor("wq", [C, GW], BF16, kind="ExternalInput").ap()
    wk = nc.dram_tensor("wk", [C, GW], BF16, kind="ExternalInput").ap()
    wv = nc.dram_tensor("wv", [C, GW], BF16, kind="ExternalInput").ap()
    bq = nc.dram_tensor("bq", [GW], F32, kind="ExternalInput").ap()
    wo = nc.dram_tensor("wo", [GW, C], BF16, kind="ExternalInput").ap()
    out = nc.dram_tensor("out", [T, C], F32, kind="ExternalOutput").ap()

    with tile.TileContext(nc) as tc, ExitStack() as ctx:
        const_p = ctx.enter_context(tc.tile_pool(name="const", bufs=1))
        w_p = ctx.enter_context(tc.tile_pool(name="w", bufs=1))
        x_p = ctx.enter_context(tc.tile_pool(name="x", bufs=1))
        qk_p = ctx.enter_context(tc.tile_pool(name="qk", bufs=1))
        v_p = ctx.enter_context(tc.tile_pool(name="v", bufs=1))
        y_p = ctx.enter_context(tc.tile_pool(name="y", bufs=1))
        exp_p = ctx.enter_context(tc.tile_pool(name="expt", bufs=8))
        misc_p = ctx.enter_context(tc.tile_pool(name="misc", bufs=2))
        stage_p = ctx.enter_context(tc.tile_pool(name="stage", bufs=3))
        # PSUM budget (8 banks): s_ps 2x[128,1024]=4, small_ps 4x[128,512]=4
        s_ps = ctx.enter_context(tc.tile_pool(name="s_ps", bufs=2, space="PSUM"))
        small_ps = ctx.enter_context(
            tc.tile_pool(name="small_ps", bufs=4, space="PSUM"))
        rec_p = ctx.enter_context(tc.tile_pool(name="rec", bufs=4))
        bc_pool = ctx.enter_context(tc.tile_pool(name="bc_sb", bufs=4))

        def act_recip(out_ap, in_ap):
            eng = nc.scalar
            ins = [eng.lower_ap(in_ap)]
            for val in (0.0, 1.0, 0.0):  # bias, scale, alpha
                ins.append(mybir.ImmediateValue(dtype=mybir.dt.float32,
                                                value=val))
            return eng.add_instruction(mybir.InstActivation(
                name=nc.get_next_instruction_name(),
                func=AF.Reciprocal, ins=ins, outs=[eng.lower_ap(out_ap)]))

        # ---- constants ----
        # selector for broadcasting recip rowsums via PE: row64 (head A's L)
        # -> out partitions 0:64, row63 (head B's L) -> out partitions 64:128
        sel_sb = const_p.tile([65, 128], BF16)
        nc.gpsimd.memset(sel_sb[:], 0.0)
        nc.gpsimd.memset(sel_sb[64:65, 0:64], 1.0)
        nc.gpsimd.memset(sel_sb[32:33, 64:128], 1.0)
        # causal 0/1 mask with column offset 512: mask[p, j] = 1 iff j-512 >= p.
        # Diagonal strip r uses slice [512 - r*128, 1024 - r*128):
        # mask[p, 512 - r*128 + jq] = 1 iff jq >= r*128 + p.
        mask_sb = const_p.tile([128, 1024], BF16)
        nc.gpsimd.memset(mask_sb[:], 1.0)
        nc.gpsimd.affine_select(
            out=mask_sb[:], in_=mask_sb[:], compare_op=ALU.is_ge,
            fill=0.0, base=-512, pattern=[[1, 1024]], channel_multiplier=-1)


        # ---- weight + input loads ----
        # Interleaved per-cc and split along t so the first projection
        # matmul (needs wq[cc0] + xT[cc0, 0:512] only) unblocks within a
        # few microseconds instead of waiting for all ~11 MB of loads.
        wq_sb = w_p.tile([128, CC, GW], BF16)
        wk_sb = w_p.tile([128, CC, GW], BF16)
        wv_sb = w_p.tile([128, CC, GW], BF16)
        xT_sb = x_p.tile([128, CC, T], BF16)
        for cc in range(CC):
            csl = slice(cc * 128, (cc + 1) * 128)
            # spread the critical-path chunks (wq + xT feed the first
            # projections) across BOTH HWDGE issue queues
            (nc.sync if cc % 2 == 0 else nc.scalar).dma_start(
                wq_sb[:, cc, :], wq[csl, :])
            for tc_ in range(QC):
                tsl = slice(tc_ * 512, (tc_ + 1) * 512)
                (nc.sync if tc_ % 2 == 0 else nc.scalar).dma_start(
                    xT_sb[:, cc, tsl], xT[csl, tsl])
            (nc.scalar if cc % 2 == 0 else nc.sync).dma_start(
                wk_sb[:, cc, :], wk[csl, :])
            (nc.scalar if cc % 2 == 0 else nc.sync).dma_start(
                wv_sb[:, cc, :], wv[csl, :])
        wo_sb = w_p.tile([128, PAIRS, C], BF16)
        for jp in range(PAIRS):
            nc.scalar.dma_start(wo_sb[:, jp, :], wo[jp * 128:(jp + 1) * 128, :])
        bq_sb = w_p.tile([128, PAIRS], F32)
        nc.sync.dma_start(bq_sb[:], bq.rearrange("(p c) -> c p", c=128))

        # ---- Q^T / K^T projections: qT[pair] = [128 (2 heads x d), T] ----
        qT = [qk_p.tile([128, T], BF16, tag=f"qT{p}", name=f"qT{p}") for p in range(PAIRS)]
        kT = [qk_p.tile([128, T], BF16, tag=f"kT{p}", name=f"kT{p}") for p in range(PAIRS)]
        for p in range(PAIRS):
            jsl = slice(p * 128, (p + 1) * 128)
            for w_sb, dst, biased in ((wq_sb, qT[p], True), (wk_sb, kT[p], False)):
                # cc-outer with 4 live psum banks so the stationary (weights)
                # is reused across 4 matmuls
                pss = [small_ps.tile([128, 512], F32, tag="sm",
                                     name=f"pp{p}{biased}{i}") for i in range(QC)]
                for cc in range(CC):
                    for t in range(QC):
                        nc.tensor.matmul(pss[t][:], w_sb[:, cc, jsl],
                                         xT_sb[:, cc, t * 512:(t + 1) * 512],
                                         start=(cc == 0), stop=(cc == CC - 1))
                for t in range(QC):
                    tsl = slice(t * 512, (t + 1) * 512)
                    if biased:
                        nc.scalar.activation(dst[:, tsl], pss[t][:],
                                             AF.Identity,
                                             bias=bq_sb[:, p:p + 1])
                    else:
                        nc.scalar.copy(dst[:, tsl], pss[t][:])

        # ---- V projection into ones-augmented layout ----
        # v_sb [128 (tk), tt, h, 128]:
        #   even head (A): cols 0:64 = V, col 64 = 1, rest 0   -> lhsT [.., 0:65]
        #   odd head (B):  col 32 = 1, cols 64:128 = V, rest 0 -> lhsT [.., 0:128]
        # The P@V matmul then lands y_A in psum partitions 0:64 + L_A at 64,
        # and y_B at 64:128 + L_B at 32 (zero-pad columns cost nothing extra:
        # matmul time is set by the streamed N, not M; L_B sits at 32 so the
        # DVE reciprocal ops stay 32-partition-aligned).
        v_sb = v_p.tile([128, TT, HPG, 128], BF16)
        nc.gpsimd.memset(v_sb[:], 0.0)
        for h in range(HPG):
            col = 64 if h % 2 == 0 else 32
            nc.gpsimd.memset(v_sb[:, :, h, col:col + 1], 1.0)
        for tt in range(TT):
            ps = small_ps.tile([128, 512], F32, tag="sm")
            for cc in range(CC):
                nc.tensor.matmul(ps[:], xT_sb[:, cc, tt * 128:(tt + 1) * 128],
                                 wv_sb[:, cc, :],
                                 start=(cc == 0), stop=(cc == CC - 1))
            psv = ps.rearrange("p (h d) -> p h d", d=D)
            nc.scalar.copy(v_sb[:, tt, 0:HPG:2, 0:D], psv[:, 0:HPG:2, :])
            nc.scalar.copy(v_sb[:, tt, 1:HPG:2, 64:128], psv[:, 1:HPG:2, :])

        # ---- attention + out-projection, per query chunk ----
        yT = [y_p.tile([128, T], BF16, tag=f"yT{p}", name=f"yT{p}") for p in range(PAIRS)]
        for qc in range(QC):
            qsl = slice(qc * 512, (qc + 1) * 512)
            for p in range(PAIRS):
                hA, hB = 2 * p, 2 * p + 1
                nki = 4 * qc + 4
                ya_ps = small_ps.tile([128, 512], F32, tag="sm")
                yb_ps = small_ps.tile([128, 512], F32, tag="sm")
                for ki in range(nki):
                    ksl = slice(ki * 128, (ki + 1) * 128)
                    r = ki - 4 * qc
                    off = max(0, r) * 128  # first valid tq col of this strip
                    qv = slice(qc * 512 + off, (qc + 1) * 512)
                    sps = s_ps.tile([128, 1024], F32, tag="s")
                    nc.tensor.matmul(sps[:, off:512], kT[p][0:64, ksl],
                                     qT[p][0:64, qv], start=True, stop=True,
                                     tile_position=(0, 0))
                    nc.tensor.matmul(sps[:, 512 + off:1024], kT[p][64:128, ksl],
                                     qT[p][64:128, qv], start=True, stop=True,
                                     tile_position=(64, 0))
                    et = exp_p.tile([128, 1024], BF16, tag="e")
                    if off == 0:
                        nc.scalar.activation(et[:], sps[:], AF.Exp, scale=SCALE)
                    else:
                        # one ACT op covering both halves' valid columns via a
                        # two-segment strided AP (saves the per-op fixed cost)
                        e2 = et.rearrange("p (h w) -> p h w", h=2)[:, :, off:512]
                        s2 = sps.rearrange("p (h w) -> p h w", h=2)[:, :, off:512]
                        nc.scalar.activation(e2, s2, AF.Exp, scale=SCALE)
                    if r >= 0:
                        # triangular boundary lives in cols [off, off+128);
                        # cols beyond off+128 are fully valid
                        msl = mask_sb[:, 512:640]
                        nc.vector.tensor_mul(et[:, off:off + 128],
                                             et[:, off:off + 128], msl)
                        nc.vector.tensor_mul(et[:, 512 + off:640 + off],
                                             et[:, 512 + off:640 + off], msl)
                    mmkw = dict(start=(ki == 0), stop=(ki == nki - 1),
                                skip_group_check=True)
                    nc.tensor.matmul(ya_ps[:, off:512],
                                     v_sb[:, ki, hA, :],
                                     et[:, off:512], tile_position=(0, 0),
                                     **mmkw)
                    nc.tensor.matmul(yb_ps[:, off:512], v_sb[:, ki, hB, :],
                                     et[:, 512 + off:1024],
                                     tile_position=(0, 0), **mmkw)
                # Release the y psum banks FAST (raw copies + L-row copies),
                # then normalize yT in place off the critical path. The two
                # K=1 selector matmuls broadcast L_A to partitions 0:64 and
                # L_B to 64:128 reading only the rows we wrote.
                rec_t = rec_p.tile([65, 512], BF16, tag="rec")
                nc.vector.tensor_copy(rec_t[64:65, :], ya_ps[64:65, :])
                nc.vector.tensor_copy(rec_t[32:33, :], yb_ps[32:33, :])
                nc.vector.tensor_copy(yT[p][0:64, qsl], ya_ps[0:64, :])
                nc.vector.tensor_copy(yT[p][64:128, qsl], yb_ps[64:128, :])
                bc_ps = small_ps.tile([128, 512], F32, tag="sm")
                nc.tensor.matmul(bc_ps[:], sel_sb[64:65, :], rec_t[64:65, :],
                                 start=True, stop=False, skip_group_check=True)
                nc.tensor.matmul(bc_ps[:], sel_sb[32:33, :], rec_t[32:33, :],
                                 start=False, stop=True, skip_group_check=True)
                bc_sb = bc_pool.tile([128, 512], F32, tag="bc")
                nc.vector.reciprocal(bc_sb[:], bc_ps[:])
                nc.vector.tensor_mul(yT[p][0:64, qsl], yT[p][0:64, qsl],
                                     bc_sb[0:64, :])
                nc.vector.tensor_mul(yT[p][64:128, qsl], yT[p][64:128, qsl],
                                     bc_sb[64:128, :])
            # out-projection for this query chunk
            for tt in range(4 * qc, 4 * qc + 4):
                st = stage_p.tile([128, 1024], F32, tag="st")
                for nck in range(2):
                    nsl = slice(nck * 512, (nck + 1) * 512)
                    ops = small_ps.tile([128, 512], F32, tag="sm")
                    for jp in range(PAIRS):
                        nc.tensor.matmul(
                            ops[:], yT[jp][:, tt * 128:(tt + 1) * 128],
                            wo_sb[:, jp, nsl],
                            start=(jp == 0), stop=(jp == PAIRS - 1))
                    nc.vector.tensor_copy(st[:, nsl], ops[:])
                nc.sync.dma_start(out[tt * 128:(tt + 1) * 128, :], st[:])

    nc.compile()
    return nc


_NC_CACHE = None


def _get_nc():
    global _NC_CACHE
    if _NC_CACHE is None:
        _NC_CACHE = build_kernel()
    return _NC_CACHE


def _shard(x, w_qkv, b_qkv, w_out, b_out):
    """Build per-core input maps. Core c: batch c//2, head-group c%2."""
    bf = ml_dtypes.bfloat16
    in_maps = []
    for c in range(N_CORES):
        b, g = divmod(c, 2)
        gs = slice(g * GW, g * GW + GW)
        in_maps.append({
            "xT": np.ascontiguousarray(x[b].T).astype(bf),
            "wq": np.ascontiguousarray(w_qkv[:, gs]).astype(bf),
            "wk": np.ascontiguousarray(w_qkv[:, C + g * GW:C + g * GW + GW]).astype(bf),
            "wv": np.ascontiguousarray(w_qkv[:, 2 * C + g * GW:2 * C + g * GW + GW]).astype(bf),
            "bq": np.ascontiguousarray(b_qkv[gs]).astype(np.float32),
            "wo": np.ascontiguousarray(w_out[g * GW:g * GW + GW, :]).astype(bf),
        })
    return in_maps


def _unshard(results, b_qkv, w_out, b_out):
    # host-side constant: b_v @ w_out rows (exact: softmax rows sum to 1)
    bv = b_qkv[2 * C:3 * C].astype(np.float64)
    const_row = (bv @ w_out.astype(np.float64)) + b_out.astype(np.float64)
    out = np.empty((B, T, C), dtype=np.float32)
    for b in range(B):
        acc = (results[2 * b]["out"].astype(np.float64)
               + results[2 * b + 1]["out"].astype(np.float64) + const_row)
        out[b] = acc.astype(np.float32)
    return out


def _run(in_maps, trace=False, tmpdir=None):
    nc = _get_nc()
    return run_bass_kernel_spmd(nc, in_maps, core_ids=list(range(N_CORES)),
                                trace=trace, tmpdir=tmpdir)


def kernel(x, w_qkv, b_qkv, w_out, b_out):
    x = np.asarray(x, dtype=np.float32)
    w_qkv = np.asarray(w_qkv, dtype=np.float32)
    b_qkv = np.asarray(b_qkv, dtype=np.float32)
    w_out = np.asarray(w_out, dtype=np.float32)
    b_out = np.asarray(b_out, dtype=np.float32)
    res = _run(_shard(x, w_qkv, b_qkv, w_out, b_out))
    return _unshard(res.results, b_qkv, w_out, b_out)


def kernel_profiled(x, w_qkv, b_qkv, w_out, b_out, tmpdir=None):
    """Like kernel() but captures an NTFF profile (requires the NTFF hook
    to be registered, e.g. via prof_shim.install()). Returns (out, result)."""
    if tmpdir is None:
        tmpdir = tempfile.mkdtemp(prefix="attn_trace_")
    x = np.asarray(x, dtype=np.float32)
    w_qkv = np.asarray(w_qkv, dtype=np.float32)
    b_qkv = np.asarray(b_qkv, dtype=np.float32)
    w_out = np.asarray(w_out, dtype=np.float32)
    b_out = np.asarray(b_out, dtype=np.float32)
    res = _run(_shard(x, w_qkv, b_qkv, w_out, b_out), trace=True,
               tmpdir=tmpdir)
    return _unshard(res.results, b_qkv, w_out, b_out), res
